# revision 54
# baseline (speedup 1.0000x reference)
"""Trainium2 Bass kernel for nn_AdapterBlock (LN -> dwconv x3 -> SE -> residual).

Data-parallel over batch: 8 samples -> 8 NeuronCores. v6: readiness-ordered
emission (all engine queues are in-order, so emission order IS the schedule).

Per core:
  - x loads f32 via the sync-engine HWDGE queue (~330 GB/s) into a 4-slot
    staging ring; the f32->bf16 cast is folded into the LN apply. The old
    casting SWDGE path ran at ~150 GB/s and gated the whole kernel.
  - LN stats per t-tile split ACT(accum)/DVE(bn_stats); apply split
    ACT(scale/bias form)/DVE(tensor_scalar) by tile parity, writing
    normalized bf16 into zX
  - xbar DMA-transpose to layout B per 4-tile group; scatter per (group,
    channel) so dep ranges stay inside one channel's row of xB
  - conv1 windows [0,510,1022,1534,2046): window q only needs groups <= q,
    so conv1 stage q is emitted right after scatter(q); final 2 columns
    patched by tiny DVE ops. Channel engines CH_ENG: 'P' diag-matmul on PE
    (evac split ACT/DVE by channel parity), 'H' chain on DVE
  - ~K_WARM dummy matmuls before conv1 warm the PE HAM clock gate
    (cold PE runs at half clock for 3.4us after any >3.4us idle)
  - conv23 on PE, k-outer over quarter-pairs in one 2-bank PSUM tile:
    7 LDWEIGHTS per channel-pair instead of 28, one paired evacuation
    (ACT, accum_out feeds SE pool) into contiguous c3h halves
  - one B->A transpose per half (DMA_TRANSPOSE has ~2us fixed cost)
  - gate broadcast: replicate h over 128 cols, single matmul pair against
    fc2 [H, C] bf16, sigmoid -> gateA
  - tail: out = c3*gateA + residual per 2-tile chunk on DVE; bf16 HWDGE
    store on scalar/sync queues, host upcasts to f32
"""

import os
import sys

sys.path.insert(0, "/opt/trn_rl_repo")

from contextlib import ExitStack

import numpy as np

import concourse.bass as bass  # noqa: F401
import concourse.bacc as bacc
import concourse.tile as tile
import concourse.mybir as mybir
from concourse.bass_utils import run_bass_kernel_spmd

B, T, C = 8, 2048, 1024
N_CORES = 8
NT = T // 128          # 16 t-tiles
NCH = C // 128         # 8 channel groups
H = C // 16            # SE hidden = 64
PAD = 4                # zero pad each side of the time axis (>= conv halo 3)
TF = T + 2 * PAD
QT = 512               # conv quarter
EPS = 1e-5

# conv1 windows: window q reads cols [W1[q]-1, W1[q+1]+1), chosen so it only
# needs scatter groups <= q; the last 2 columns are patched separately
W1 = [0, 510, 1022, 1534, 2046]
W23 = [0, 512, 1024, 1536, 2048]

F32 = mybir.dt.float32
BF16 = mybir.dt.bfloat16
AF = mybir.ActivationFunctionType
OP = mybir.AluOpType

# --- tunables ------------------------------------------------------------
# conv1 engine per channel-group: 'P' = TensorE diag-matmul, 'H' = DVE
# chain. conv23 is always PE. P channels must form a prefix.
CH_ENG = os.environ.get("K_CH_ENG", "PPPPPPPP")
STATS_ACT = set(int(x) for x in
                os.environ.get("K_STATS_ACT", "0,2,4,6,8,10,12,14").split(",")
                if x != "")
APPLY_ACT = set(int(x) for x in
                os.environ.get("K_APPLY_ACT", "0,2,4,6,8,10,12,14").split(",")
                if x != "")
K_WARM = int(os.environ.get("K_WARM", "12"))
K_BRIDGE = int(os.environ.get("K_BRIDGE", "4"))
TAIL_ENG = os.environ.get("K_TAIL_ENG", "VVVVVVVV")
N_P = len(CH_ENG) - len(CH_ENG.lstrip('P'))  # leading P channels

# measured x-tile arrival times (us): sync queue t0-3, scalar t4-7 (after
# small weights), gpsimd cast t8-15
X_ARRIVE = [13, 15, 19, 21, 22, 24, 28, 30,
            10, 12, 15, 17, 20, 23, 26, 29]

_CACHE = {}


def _build():
    nc = bacc.Bacc("TRN2", target_bir_lowering=False, debug=False,
                   num_devices=N_CORES)

    x_ext = nc.dram_tensor("x", [T, C], F32, kind="ExternalInput").ap()
    res_ext = nc.dram_tensor("res", [T, C], F32, kind="ExternalInput").ap()
    w1_ext = nc.dram_tensor("w1p", [128, NCH, 3], F32, kind="ExternalInput").ap()
    b1_ext = nc.dram_tensor("b1p", [128, NCH], F32, kind="ExternalInput").ap()
    ec_ext = nc.dram_tensor("ecp", [128, NCH, 4], F32, kind="ExternalInput").ap()
    w23_ext = nc.dram_tensor("w23p", [128, NCH, 7], F32, kind="ExternalInput").ap()
    dmask_ext = nc.dram_tensor("dmask", [128, 128], BF16, kind="ExternalInput").ap()
    fc1_ext = nc.dram_tensor("fc1p", [128, NCH, H], F32, kind="ExternalInput").ap()
    fc2_ext = nc.dram_tensor("fc2p", [H, NCH * 128], BF16, kind="ExternalInput").ap()
    out_ext = nc.dram_tensor("out", [T, C], BF16, kind="ExternalOutput").ap()

    x_src = x_ext.rearrange("(th p) c -> p th c", p=128)
    res_src = res_ext.rearrange("(th p) c -> p th c", p=128)
    out_dst = out_ext.rearrange("(th p) c -> p th c", p=128)

    with tile.TileContext(nc) as tc, ExitStack() as ctx:
        pool = ctx.enter_context(tc.tile_pool(name="main", bufs=1))
        from concourse.tile_rust import add_dep_helper

        # ---- weights (scalar HWDGE queue; d1 first, conv1 needs it ~13us)
        w1sb = pool.tile([128, NCH, 3], F32, tag="w1sb")
        b1sb = pool.tile([128, NCH], F32, tag="b1sb")
        ecsb = pool.tile([128, NCH, 4], F32, tag="ecsb")
        d1sb = pool.tile([128, 3, NCH, 128], BF16, tag="d1sb")
        fc1sb = pool.tile([128, NCH, H], F32, tag="fc1sb")
        fc2sb = pool.tile([H, NCH * 128], BF16, tag="fc2sb")
        d23sb = pool.tile([128, 7, NCH, 128], BF16, tag="d23sb")
        w23sb = pool.tile([128, NCH, 7], F32, tag="w23sb")
        dmask = pool.tile([128, 128], BF16, tag="dmask")


        # ---- buffers ----
        zX = pool.tile([128, NT, C], BF16, tag="zX")
        scr = pool.tile([128, C], BF16, tag="scr")
        sums = pool.tile([128, NT], F32, tag="sums")
        sumsq = pool.tile([128, NT], F32, tag="sumsq")
        mu = pool.tile([128, NT], F32, tag="mu")
        rstd = pool.tile([128, NT], F32, tag="rstd")
        negmr = pool.tile([128, NT], F32, tag="negmr")
        varv = pool.tile([128, NT], F32, tag="varv")
        epsb = pool.tile([128, 1], F32, tag="epsb")
        nc.vector.memset(epsb[:], EPS)
        # f32 x staging for tiles 0-7 (sync HWDGE loads in escalating chunk
        # sizes; no ring so the in-order sync queue never stalls on reuse).
        # Tiles 8-15 arrive via gpsimd cast-SWDGE straight into zX as bf16.
        xstg07 = pool.tile([128, 8, C], F32, tag="xstg07")
        # A->B stage ring; scatter trails one group so depth 2 suffices
        stgab = [pool.tile([128, 4 * C], BF16, tag="sab", name=f"sab{g}",
                           bufs=2)
                 for g in range(4)]
        stgba = [pool.tile([128, 8 * C], BF16, tag=f"sba{i}", name=f"sba{i}")
                 for i in range(2)]

        def sab(g):  # A->B view for 4-tile group g: [p, th(4), ch, t(128)]
            return stgab[g][:].rearrange("p (th ch t) -> p th ch t",
                                         th=4, ch=NCH)

        def sba(h):  # B->A view of half h: [p, ch, th(8), c(128)]
            return stgba[h][:].rearrange("p (ch th c) -> p ch th c",
                                         ch=NCH, th=8)
        xB = pool.tile([128, NCH, TF], BF16, tag="xB")
        nc.vector.memset(xB[:, :, 0:PAD], 0.0)
        nc.vector.memset(xB[:, :, PAD + T:TF], 0.0)
        rall = pool.tile([128, NCH, TF], BF16, tag="rall")
        nc.vector.memset(rall[:, :, 0:PAD], 0.0)
        nc.vector.memset(rall[:, :, PAD + T:TF], 0.0)
        pools = pool.tile([128, NCH, 2], F32, tag="pools")
        gateA = pool.tile([128, C], BF16, tag="gateA")
        h_rep = pool.tile([H, 128], BF16, tag="h_rep")
        ones_h = pool.tile([H, 128], BF16, tag="ones_h")
        nc.vector.memset(ones_h[:], 1.0)

        psum = ctx.enter_context(tc.tile_pool(name="ps", bufs=2, space="PSUM"))

        # ---- x loads + weights: each DMA queue tops out ~130-180 GB/s, so
        # x is spread over all three (sync f32 / scalar f32 / gpsimd cast)
        nc.sync.dma_start(xstg07[:, 0, :], x_src[:, 0, :])
        nc.sync.dma_start(xstg07[:, 1, :], x_src[:, 1, :])
        nc.sync.dma_start(xstg07[:, 2:4, :], x_src[:, 2:4, :])
        # tiles 8-15: cast f32->bf16 straight into zX on gpsimd SWDGE
        for c in range(4):
            t0 = 8 + 2 * c
            nc.gpsimd.dma_start(zX[:, t0:t0 + 2, :], x_src[:, t0:t0 + 2, :])
        # small weights first on scalar (the conv weights are built on-device
        # from these; the old 2.6MB diag-matrix loads ate the HBM ingest that
        # x needs in the first 30us)
        nc.scalar.dma_start(dmask[:], dmask_ext)
        nc.scalar.dma_start(w1sb[:], w1_ext)
        nc.scalar.dma_start(b1sb[:], b1_ext)
        nc.scalar.dma_start(w23sb[:], w23_ext)
        nc.scalar.dma_start(ecsb[:], ec_ext)
        nc.scalar.dma_start(xstg07[:, 4:6, :], x_src[:, 4:6, :])
        nc.scalar.dma_start(fc1sb[:], fc1_ext)
        nc.scalar.dma_start(fc2sb[:], fc2_ext)
        nc.scalar.dma_start(xstg07[:, 6:8, :], x_src[:, 6:8, :])

        # conv1 diag weights: d1[:, k, ch, :] = diag(w1[ch-block, k]);
        # high priority so the scheduler runs them on the idle early DVE
        # ahead of phase-A stats (they gate the whole PE stream)
        with tc.high_priority():
            for k in range(3):
                for ch in range(NCH):
                    nc.vector.tensor_scalar(d1sb[:, k, ch, :], dmask[:],
                                            w1sb[:, ch, k:k + 1], None,
                                            OP.mult)
        # keep the PE HAM clock gate warm before conv1 arrives; reads d1sb
        # (first DMA, lands ~3us), writes a throwaway psum tile
        wps = psum.tile([128, 128], F32, tag="warm", name="warm")
        if K_WARM and N_P >= 4:
            wrhs = d1sb[:, 0, :, :].rearrange("p a b -> p (a b)")[:, 0:QT]
            for i in range(K_WARM):
                nc.tensor.matmul(wps[:], d1sb[:, 0, 0, :], wrhs[:, 0:128],
                                 start=True, stop=True)

        def bridge_mm(t):
            # throwaway matmul gated on apply(t): keeps the PE HAM activity
            # window alive through the apply->transpose->scatter latency so
            # the clock gate doesn't drop back to half rate (~100ns each)
            nc.tensor.matmul(wps[:], d1sb[:, 0, 0, :], zX[:, t, 0:128],
                             start=True, stop=True)

        # ---- phase helpers ----
        def xf_src(t):
            return xstg07[:, t, :] if t < 8 else zX[:, t, :]

        def stats(t):
            xf = xf_src(t)
            if t in STATS_ACT:
                nc.scalar.activation(scr[:], xf, AF.Copy,
                                     accum_out=sums[:, t:t + 1])
                nc.scalar.activation(scr[:], xf, AF.Square,
                                     accum_out=sumsq[:, t:t + 1])
                nc.vector.tensor_scalar_mul(mu[:, t:t + 1],
                                            sums[:, t:t + 1], 1.0 / C)
                nc.vector.tensor_tensor(varv[:, t:t + 1], mu[:, t:t + 1],
                                        mu[:, t:t + 1], op=OP.mult)
                nc.vector.scalar_tensor_tensor(varv[:, t:t + 1],
                                               sumsq[:, t:t + 1],
                                               1.0 / C, varv[:, t:t + 1],
                                               OP.mult, OP.subtract)
            else:
                bs = pool.tile([128, 2, 6], F32, tag="bstats",
                               name=f"bs_{t}", bufs=4)
                nc.vector.bn_stats(bs[:, 0, :], xf[:, 0:512])
                nc.vector.bn_stats(bs[:, 1, :], xf[:, 512:1024])
                agg = pool.tile([128, 2], F32, tag="agg",
                                name=f"agg_{t}", bufs=4)
                nc.vector.bn_aggr(agg[:], bs[:])
                nc.vector.tensor_copy(mu[:, t:t + 1], agg[:, 0:1])
                nc.vector.tensor_copy(varv[:, t:t + 1], agg[:, 1:2])

        def finish_group_stats(g):
            gs = slice(4 * g, 4 * g + 4)
            nc.scalar.activation(varv[:, gs], varv[:, gs], AF.Sqrt,
                                 bias=epsb[:])
            nc.vector.reciprocal(rstd[:, gs], varv[:, gs])
            nc.vector.scalar_tensor_tensor(negmr[:, gs], mu[:, gs], -1.0,
                                           rstd[:, gs], OP.mult, OP.mult)

        def apply_ln(t):
            if t in APPLY_ACT:
                nc.scalar.activation(zX[:, t, :], xf_src(t), AF.Identity,
                                     scale=rstd[:, t:t + 1],
                                     bias=negmr[:, t:t + 1])
            else:
                nc.vector.tensor_scalar(zX[:, t, :], xf_src(t),
                                        mu[:, t:t + 1], rstd[:, t:t + 1],
                                        OP.subtract, OP.mult)

        xpose = [None] * 4

        def emit_scatter(g):
            for ch in range(NCH):
                dst = xB[:, ch, PAD + g * 512:PAD + (g + 1) * 512].rearrange(
                    "p (th t) -> p th t", th=4)
                src = sab(g)[:, :, ch, :]
                # on DVE (Pool's Q7 copy path measured ~6x slower); the ACT
                # queue must not sit between the transpose and the PE stream
                cp = nc.vector.tensor_copy(dst, src)
                add_dep_helper(cp.ins, xpose[g].ins, reason="xbar ordering")

        # ---- conv helpers ----
        def rsl(ch, a, b):
            return rall[:, ch, a:b]

        def conv1_pe(ch, q):
            lo, hi = W1[q], W1[q + 1]
            n = hi - lo
            ps1 = psum.tile([128, QT], F32, tag="c1ps", name=f"c1ps_{ch}_{q}")
            for k in range(3):
                off = PAD - 1 + k + lo
                nc.tensor.matmul(ps1[:, 0:n], d1sb[:, k, ch, :],
                                 xB[:, ch, off:off + n],
                                 start=(k == 0), stop=(k == 2))
            if ch % 2 == 0:
                nc.scalar.activation(rsl(ch, PAD + lo, PAD + hi), ps1[:, 0:n],
                                     AF.Relu, bias=b1sb[:, ch:ch + 1])
            else:
                nc.vector.tensor_scalar(rsl(ch, PAD + lo, PAD + hi),
                                        ps1[:, 0:n], b1sb[:, ch:ch + 1], 0.0,
                                        OP.add, OP.max)

        def conv1_lastcols(ch):
            # final 2 columns t=2046,2047 for PE channels (tiny DVE ops)
            sa = rsl(ch, PAD + 2046, PAD + 2048)
            xs = lambda d: xB[:, ch, PAD + 2046 + d:PAD + 2048 + d]
            nc.vector.tensor_scalar(sa, xs(-1), w1sb[:, ch, 0:1], None,
                                    OP.mult)
            nc.vector.scalar_tensor_tensor(sa, xs(0), w1sb[:, ch, 1:2], sa,
                                           OP.mult, OP.add)
            nc.vector.scalar_tensor_tensor(sa, xs(1), w1sb[:, ch, 2:3], sa,
                                           OP.mult, OP.add)
            nc.vector.tensor_scalar(sa, sa, b1sb[:, ch:ch + 1], 0.0,
                                    OP.add, OP.max)

        def conv1_chain(ch, q):
            lo = W1[q]
            hi = 2048 if q == 3 else W1[q + 1]
            a = rsl(ch, PAD + lo, PAD + hi)
            xs = lambda d: xB[:, ch, PAD + lo + d:PAD + hi + d]
            nc.vector.tensor_scalar(a, xs(-1), w1sb[:, ch, 0:1],
                                    b1sb[:, ch:ch + 1], OP.mult, OP.add)
            nc.vector.scalar_tensor_tensor(a, xs(0), w1sb[:, ch, 1:2], a,
                                           OP.mult, OP.add)
            nc.vector.scalar_tensor_tensor(a, xs(1), w1sb[:, ch, 2:3], a,
                                           OP.mult, OP.add)
            nc.vector.tensor_scalar_max(a, a, 0.0)

        def conv1_any(ch, q):
            if CH_ENG[ch] == 'P':
                conv1_pe(ch, q)
            else:
                conv1_chain(ch, q)

        d23_built = set()

        def build_d23(ch):
            if ch in d23_built:
                return
            d23_built.add(ch)
            # build this channel's 7 diag tap matrices on-device; ACT for
            # even channels, DVE for odd (both have slack mid-conv)
            for k in range(7):
                if ch % 2 == 0:
                    nc.scalar.activation(d23sb[:, k, ch, :], dmask[:],
                                         AF.Identity,
                                         scale=w23sb[:, ch, k:k + 1])
                else:
                    nc.vector.tensor_scalar(d23sb[:, k, ch, :], dmask[:],
                                            w23sb[:, ch, k:k + 1], None,
                                            OP.mult)

        def conv23_pair(ch, hh):
            # quarter pair (2hh, 2hh+1) in one 2-bank psum tile; k-outer so
            # each LDWEIGHTS feeds two 512-col matmuls
            build_d23(ch)
            qa, qb = 2 * hh, 2 * hh + 1
            pss = psum.tile([128, 1024], F32, tag="c23ps",
                            name=f"c23ps_{ch}_{hh}")
            for k in range(7):
                for j, q in enumerate((qa, qb)):
                    lo = W23[q]
                    off = PAD - 3 + k + lo
                    nc.tensor.matmul(pss[:, j * QT:(j + 1) * QT],
                                     d23sb[:, k, ch, :],
                                     rsl(ch, off, off + QT),
                                     start=(k == 0), stop=(k == 6))
            # edge-fix for the last column (hh==1): the padded 7-tap
            # composite differs from conv3(conv2(.)) there; must patch in
            # PSUM since this half's transpose follows immediately. The
            # hh==0 edge (t=0) is patched batched across channels later to
            # avoid a per-pair DVE-waits-PE stall.
            if hh == 1:
                e = pss[:, 1023:1024]
                nc.vector.scalar_tensor_tensor(e, rsl(ch, PAD + T - 2,
                                                      PAD + T - 1),
                                               ecsb[:, ch, 2:3], e,
                                               OP.mult, OP.add)
                nc.vector.scalar_tensor_tensor(e, rsl(ch, PAD + T - 1,
                                                      PAD + T),
                                               ecsb[:, ch, 3:4], e,
                                               OP.mult, OP.add)
            # c3 aliases xB (conv1 reads of the overlapped columns are
            # emitted earlier, so the WAR ordering holds)
            nc.scalar.activation(
                xB[:, ch, PAD + 1024 * hh:PAD + 1024 * (hh + 1)],
                pss[:], AF.Copy, accum_out=pools[:, ch, hh:hh + 1])
            if hh == 1:
                nc.sync.dma_start(
                    out=sba(hh)[:, ch, :, :],
                    in_=xB[:, ch, PAD + 1024 * hh:PAD + 1024 * (hh + 1)],
                    transpose=True)

        def edge_fix0():
            # batched t=0 edge patch for all channels (2 multiply + 2 add
            # ops instead of 16 per-pair PSUM patches); runs after all
            # pair-0 evacuations. The SE pool misses this ~1e-3 correction,
            # which shifts the sigmoid gate negligibly.
            dst = xB[:, :, PAD:PAD + 1]
            for j, c0 in enumerate((PAD, PAD + 1)):
                tmp = pool.tile([128, NCH, 1], F32, tag="efix",
                                name=f"efix0_{j}", bufs=2)
                nc.vector.tensor_tensor(tmp[:], rall[:, :, c0:c0 + 1],
                                        ecsb[:, :, j:j + 1], op=OP.mult)
                nc.vector.tensor_tensor(dst, dst, tmp[:], op=OP.add)

        def back_xpose0():
            for ch in range(NCH):
                nc.sync.dma_start(out=sba(0)[:, ch, :, :],
                                  in_=xB[:, ch, PAD:PAD + 1024],
                                  transpose=True)

        P_FIRST = sorted(range(NCH), key=lambda c: CH_ENG[c] != 'P')

        # ---- phase A + conv stream, readiness-ordered: scatter/conv for
        # group g-1 are emitted BEFORE group g's stats so the in-order ACT/
        # DVE queues run them as soon as their deps land, not after the next
        # group's x arrives ----
        for g in range(4):
            if g > 0:
                # high priority: when the scheduler has both these and the
                # next group's stats ready, feeding the PE wins
                with tc.high_priority(offset=4000):
                    emit_scatter(g - 1)
                    for ch in range(NCH):
                        conv1_any(ch, g - 1)
            if g == 3:
                # pair-0 conv23 overlaps the tail of phase A: channel ch only
                # needs its own conv1 q0-q2
                with tc.high_priority(offset=4000):
                    for ch in P_FIRST:
                        if CH_ENG[ch] == 'P':
                            conv23_pair(ch, 0)
            for t in range(4 * g, 4 * g + 4):
                # wait hint: the scheduler's DMA model thinks x lands much
                # earlier than it really does; without these it fills ACT/DVE
                # with later groups' stats ahead of the latency-critical
                # apply->transpose->scatter chain. Values ~= measured arrival.
                with tc.tile_wait_until(X_ARRIVE[t] / 1000):
                    stats(t)
            finish_group_stats(g)
            for t in range(4 * g, 4 * g + 4):
                apply_ln(t)
            if g > 0:
                with tc.high_priority(offset=4000):
                    for t in range(4 * g, 4 * g + min(K_BRIDGE, 4)):
                        bridge_mm(t)
            xpose[g] = nc.sync.dma_start(
                out=sab(g),
                in_=zX[:, 4 * g:4 * g + 4, :].rearrange("p a b -> p (a b)"),
                transpose=True)
        with tc.high_priority(offset=4000):
            emit_scatter(3)
            for ch in range(NCH):
                conv1_any(ch, 3)
                if CH_ENG[ch] == 'P':
                    conv1_lastcols(ch)

        # residual in (cast f32->bf16 SWDGE; overlaps the conv phase).
        # reuses zX; per-chunk WAR dep on that group's A->B transpose only.
        for q in range(8):
            nc.gpsimd.dma_start(zX[:, q * 2:(q + 1) * 2, :],
                                res_src[:, q * 2:(q + 1) * 2, :])

        for ch in range(NCH):
            if CH_ENG[ch] != 'P':
                conv23_pair(ch, 0)
        edge_fix0()
        back_xpose0()
        for ch in P_FIRST:
            conv23_pair(ch, 1)

        # ---- SE MLP ----
        h_ps = psum.tile([128, QT], F32, tag="c1ps", name="hps")[0:H, 0:2]
        for ch in range(NCH):
            nc.tensor.matmul(h_ps, fc1sb[:, ch, :], pools[:, ch, :],
                             start=(ch == 0), stop=(ch == NCH - 1))
        h_half = pool.tile([H, 2], F32, tag="h_half")
        nc.scalar.activation(h_half[:], h_ps, AF.Relu)
        h_sb = pool.tile([H, 1], F32, tag="hsb")
        nc.vector.tensor_reduce(h_sb[:], h_half[:], mybir.AxisListType.X,
                                OP.add)
        # replicate h across 128 cols, then one matmul pair computes
        # sigmoid(fc2^T h) broadcast over all partitions
        nc.vector.tensor_scalar_mul(h_rep[:], ones_h[:], h_sb[:])
        g_ps = psum.tile([128, 1024], F32, tag="c23ps", name="gps")
        nc.tensor.matmul(g_ps[:, 0:512], h_rep[:], fc2sb[:, 0:512],
                         start=True, stop=True)
        nc.tensor.matmul(g_ps[:, 512:1024], h_rep[:], fc2sb[:, 512:1024],
                         start=True, stop=True)
        nc.scalar.activation(gateA[:], g_ps[:], AF.Sigmoid)
        gateA3 = gateA[:].rearrange("p (ch c) -> p ch c", ch=NCH)

        # ---- output: gate mult + residual add + store, per 2-tile chunk ----
        for q in range(8):
            hf, to = q // 4, (2 * q) % 8
            eng = nc.gpsimd if TAIL_ENG[q] == 'G' else nc.vector
            for t in (2 * q, 2 * q + 1):
                st = sba(hf)[:, :, t % 8, :]
                eng.tensor_tensor(st, st, gateA3, op=OP.mult)
            zt = zX[:, 2 * q:2 * q + 2, :].rearrange(
                "p th (ch c) -> p ch th c", ch=NCH)
            eng.tensor_tensor(zt, zt, sba(hf)[:, :, to:to + 2, :],
                              op=OP.add)
            dq = nc.scalar if q % 2 == 0 else nc.sync
            dq.dma_start(out_dst[:, 2 * q:2 * q + 2, :],
                         zX[:, 2 * q:2 * q + 2, :])

    nc.compile()
    return nc


def _prep_weights(ln_w, ln_b, w1, w2, w3, fc1, fc2):
    import ml_dtypes
    w1 = w1[:, 0, :].astype(np.float64)   # [C, 3]
    w2 = w2[:, 0, :].astype(np.float64)   # [C, 5]
    w3 = w3[:, 0, :].astype(np.float64)   # [C, 3]
    ln_w = ln_w.astype(np.float64)
    ln_b = ln_b.astype(np.float64)
    w1f = w1 * ln_w[:, None]
    b1 = (ln_b * w1.sum(axis=1))

    def to_plh(a):  # [C, K] -> [128, NCH, K]
        return np.ascontiguousarray(
            a.reshape(NCH, 128, -1).transpose(1, 0, 2)).astype(np.float32)

    w1p = to_plh(w1f)
    b1p = np.ascontiguousarray(b1.reshape(NCH, 128).T).astype(np.float32)
    fc1p = to_plh((fc1.astype(np.float64) / T).T)
    fc2p = np.ascontiguousarray(
        fc2.astype(np.float64).T.reshape(H, NCH * 128)).astype(
            ml_dtypes.bfloat16)

    w23 = np.stack([np.convolve(w3[c], w2[c]) for c in range(C)])  # [C, 7]
    # edge-fix coefficients (negated: they accumulate into the psum)
    ec = np.stack([-w3[:, 0] * w2[:, 3], -w3[:, 0] * w2[:, 4],
                   -w3[:, 2] * w2[:, 0], -w3[:, 2] * w2[:, 1]], axis=1)  # [C,4]
    w23p = to_plh(w23)
    ecp = to_plh(ec)

    def diags(wk):  # [C, K] -> [128, K, NCH, 128] bf16 (partition-major)
        K = wk.shape[1]
        d = np.zeros((K, NCH, 128, 128), np.float32)
        for k in range(K):
            for chh in range(NCH):
                np.fill_diagonal(d[k, chh], wk[chh * 128:(chh + 1) * 128, k])
        return np.ascontiguousarray(
            d.transpose(2, 0, 1, 3)).astype(ml_dtypes.bfloat16)

    import ml_dtypes
    return {"w1p": w1p, "b1p": b1p, "w23p": w23p, "ecp": ecp,
            "fc1p": fc1p, "fc2p": fc2p,
            "dmask": np.eye(128, dtype=ml_dtypes.bfloat16)}


def kernel(x, residual_input, ln_w, ln_b, w1, w2, w3, fc1, fc2):
    x = np.asarray(x, dtype=np.float32)
    residual_input = np.asarray(residual_input, dtype=np.float32)
    wts = _prep_weights(np.asarray(ln_w), np.asarray(ln_b),
                        np.asarray(w1), np.asarray(w2), np.asarray(w3),
                        np.asarray(fc1), np.asarray(fc2))

    if "nc" not in _CACHE:
        _CACHE["nc"] = _build()
    nc = _CACHE["nc"]

    in_maps = []
    for b in range(B):
        m = {"x": np.ascontiguousarray(x[b]),
             "res": np.ascontiguousarray(residual_input[b])}
        m.update(wts)
        in_maps.append(m)
    res = run_bass_kernel_spmd(nc, in_maps, core_ids=list(range(N_CORES)))
    out = np.stack([res.results[i]["out"] for i in range(N_CORES)], axis=0)
    return out.astype(np.float32)


# revision 55
# speedup vs baseline: 1.1691x; 1.1691x over previous
"""Trainium2 Bass kernel for nn_AdapterBlock (LN -> dwconv x3 -> SE -> residual).

Data-parallel over batch: 8 samples -> 8 NeuronCores. v6: readiness-ordered
emission (all engine queues are in-order, so emission order IS the schedule).

Per core:
  - x loads f32 via the sync-engine HWDGE queue (~330 GB/s) into a 4-slot
    staging ring; the f32->bf16 cast is folded into the LN apply. The old
    casting SWDGE path ran at ~150 GB/s and gated the whole kernel.
  - LN stats per t-tile split ACT(accum)/DVE(bn_stats); apply split
    ACT(scale/bias form)/DVE(tensor_scalar) by tile parity, writing
    normalized bf16 into zX
  - xbar DMA-transpose to layout B per 4-tile group; scatter per (group,
    channel) so dep ranges stay inside one channel's row of xB
  - conv1 windows [0,510,1022,1534,2046): window q only needs groups <= q,
    so conv1 stage q is emitted right after scatter(q); final 2 columns
    patched by tiny DVE ops. Channel engines CH_ENG: 'P' diag-matmul on PE
    (evac split ACT/DVE by channel parity), 'H' chain on DVE
  - ~K_WARM dummy matmuls before conv1 warm the PE HAM clock gate
    (cold PE runs at half clock for 3.4us after any >3.4us idle)
  - conv23 on PE, k-outer over quarter-pairs in one 2-bank PSUM tile:
    7 LDWEIGHTS per channel-pair instead of 28, one paired evacuation
    (ACT, accum_out feeds SE pool) into contiguous c3h halves
  - one B->A transpose per half (DMA_TRANSPOSE has ~2us fixed cost)
  - gate broadcast: replicate h over 128 cols, single matmul pair against
    fc2 [H, C] bf16, sigmoid -> gateA
  - tail: out = c3*gateA + residual per 2-tile chunk on DVE; bf16 HWDGE
    store on scalar/sync queues, host upcasts to f32
"""

import os
import sys

sys.path.insert(0, "/opt/trn_rl_repo")

from contextlib import ExitStack

import numpy as np

import concourse.bass as bass  # noqa: F401
import concourse.bacc as bacc
import concourse.tile as tile
import concourse.mybir as mybir
from concourse.bass_utils import run_bass_kernel_spmd

B, T, C = 8, 2048, 1024
N_CORES = 8
NT = T // 128          # 16 t-tiles
NCH = C // 128         # 8 channel groups
H = C // 16            # SE hidden = 64
PAD = 4                # zero pad each side of the time axis (>= conv halo 3)
TF = T + 2 * PAD
QT = 512               # conv quarter
EPS = 1e-5

# conv1 windows: window q reads cols [W1[q]-1, W1[q+1]+1), chosen so it only
# needs scatter groups <= q; the last 2 columns are patched separately
W1 = [0, 510, 1022, 1534, 2046]
W23 = [0, 512, 1024, 1536, 2048]

F32 = mybir.dt.float32
BF16 = mybir.dt.bfloat16
AF = mybir.ActivationFunctionType
OP = mybir.AluOpType

# --- tunables ------------------------------------------------------------
# conv1 engine per channel-group: 'P' = TensorE diag-matmul, 'H' = DVE
# chain. conv23 is always PE. P channels must form a prefix.
CH_ENG = os.environ.get("K_CH_ENG", "PPPPPPPP")
STATS_ACT = set(int(x) for x in
                os.environ.get("K_STATS_ACT", "0,2,4,6,8,10,12,14").split(",")
                if x != "")
APPLY_ACT = set(int(x) for x in
                os.environ.get("K_APPLY_ACT", "0,2,4,6,8,10,12,14").split(",")
                if x != "")
K_WARM = int(os.environ.get("K_WARM", "12"))
K_BRIDGE = int(os.environ.get("K_BRIDGE", "4"))
TAIL_ENG = os.environ.get("K_TAIL_ENG", "VVVVVVVV")
N_P = len(CH_ENG) - len(CH_ENG.lstrip('P'))  # leading P channels

# measured x-tile arrival times (us): sync queue t0-3, scalar t4-7 (after
# small weights), gpsimd cast t8-15
X_ARRIVE = [13, 15, 19, 21, 22, 24, 28, 30,
            10, 12, 15, 17, 20, 23, 26, 29]

_CACHE = {}


def _build():
    nc = bacc.Bacc("TRN2", target_bir_lowering=False, debug=False,
                   num_devices=N_CORES)

    x_ext = nc.dram_tensor("x", [T, C], F32, kind="ExternalInput").ap()
    res_ext = nc.dram_tensor("res", [T, C], F32, kind="ExternalInput").ap()
    w1_ext = nc.dram_tensor("w1p", [128, NCH, 3], F32, kind="ExternalInput").ap()
    b1_ext = nc.dram_tensor("b1p", [128, NCH], F32, kind="ExternalInput").ap()
    ec_ext = nc.dram_tensor("ecp", [128, NCH, 4], F32, kind="ExternalInput").ap()
    w23_ext = nc.dram_tensor("w23p", [128, NCH, 7], F32, kind="ExternalInput").ap()
    dmask_ext = nc.dram_tensor("dmask", [128, 128], BF16, kind="ExternalInput").ap()
    fc1_ext = nc.dram_tensor("fc1p", [128, NCH, H], F32, kind="ExternalInput").ap()
    fc2_ext = nc.dram_tensor("fc2p", [H, NCH * 128], BF16, kind="ExternalInput").ap()
    out_ext = nc.dram_tensor("out", [T, C], BF16, kind="ExternalOutput").ap()

    x_src = x_ext.rearrange("(th p) c -> p th c", p=128)
    res_src = res_ext.rearrange("(th p) c -> p th c", p=128)
    out_dst = out_ext.rearrange("(th p) c -> p th c", p=128)

    with tile.TileContext(nc) as tc, ExitStack() as ctx:
        pool = ctx.enter_context(tc.tile_pool(name="main", bufs=1))
        from concourse.tile_rust import add_dep_helper

        # ---- weights (scalar HWDGE queue; d1 first, conv1 needs it ~13us)
        w1sb = pool.tile([128, NCH, 3], F32, tag="w1sb")
        b1sb = pool.tile([128, NCH], F32, tag="b1sb")
        ecsb = pool.tile([128, NCH, 4], F32, tag="ecsb")
        d1sb = pool.tile([128, 3, NCH, 128], BF16, tag="d1sb")
        fc1sb = pool.tile([128, NCH, H], F32, tag="fc1sb")
        fc2sb = pool.tile([H, NCH * 128], BF16, tag="fc2sb")
        d23sb = pool.tile([128, 7, NCH, 128], BF16, tag="d23sb")
        w23sb = pool.tile([128, NCH, 7], F32, tag="w23sb")
        dmask = pool.tile([128, 128], BF16, tag="dmask")


        # ---- buffers ----
        zX = pool.tile([128, NT, C], BF16, tag="zX")
        scr = pool.tile([128, C], BF16, tag="scr")
        sums = pool.tile([128, NT], F32, tag="sums")
        sumsq = pool.tile([128, NT], F32, tag="sumsq")
        mu = pool.tile([128, NT], F32, tag="mu")
        rstd = pool.tile([128, NT], F32, tag="rstd")
        negmr = pool.tile([128, NT], F32, tag="negmr")
        varv = pool.tile([128, NT], F32, tag="varv")
        epsb = pool.tile([128, 1], F32, tag="epsb")
        nc.vector.memset(epsb[:], EPS)
        # f32 x staging for tiles 0-7 (sync HWDGE loads in escalating chunk
        # sizes; no ring so the in-order sync queue never stalls on reuse).
        # Tiles 8-15 arrive via gpsimd cast-SWDGE straight into zX as bf16.
        xstg07 = pool.tile([128, 8, C], F32, tag="xstg07")
        # A->B stage ring; scatter trails one group so depth 2 suffices
        stgab = [pool.tile([128, 4 * C], BF16, tag="sab", name=f"sab{g}",
                           bufs=2)
                 for g in range(4)]
        stgba = [pool.tile([128, 8 * C], BF16, tag=f"sba{i}", name=f"sba{i}")
                 for i in range(2)]

        def sab(g):  # A->B view for 4-tile group g: [p, th(4), ch, t(128)]
            return stgab[g][:].rearrange("p (th ch t) -> p th ch t",
                                         th=4, ch=NCH)

        def sba(h):  # B->A view of half h: [p, ch, th(8), c(128)]
            return stgba[h][:].rearrange("p (ch th c) -> p ch th c",
                                         ch=NCH, th=8)
        xB = pool.tile([128, NCH, TF], BF16, tag="xB")
        nc.vector.memset(xB[:, :, 0:PAD], 0.0)
        nc.vector.memset(xB[:, :, PAD + T:TF], 0.0)
        rall = pool.tile([128, NCH, TF], BF16, tag="rall")
        nc.vector.memset(rall[:, :, 0:PAD], 0.0)
        nc.vector.memset(rall[:, :, PAD + T:TF], 0.0)
        pools = pool.tile([128, NCH, 2], F32, tag="pools")
        gateA = pool.tile([128, C], BF16, tag="gateA")
        h_rep = pool.tile([H, 128], BF16, tag="h_rep")
        ones_h = pool.tile([H, 128], BF16, tag="ones_h")
        nc.vector.memset(ones_h[:], 1.0)

        psum = ctx.enter_context(tc.tile_pool(name="ps", bufs=2, space="PSUM"))

        # ---- x loads + weights: each DMA queue tops out ~130-180 GB/s, so
        # x is spread over all three (sync f32 / scalar f32 / gpsimd cast)
        nc.sync.dma_start(xstg07[:, 0, :], x_src[:, 0, :])
        nc.sync.dma_start(xstg07[:, 1, :], x_src[:, 1, :])
        nc.sync.dma_start(xstg07[:, 2:4, :], x_src[:, 2:4, :])
        # tiles 8-15: cast f32->bf16 straight into zX on gpsimd SWDGE
        for c in range(4):
            t0 = 8 + 2 * c
            nc.gpsimd.dma_start(zX[:, t0:t0 + 2, :], x_src[:, t0:t0 + 2, :])
        # small weights first on scalar (the conv weights are built on-device
        # from these; the old 2.6MB diag-matrix loads ate the HBM ingest that
        # x needs in the first 30us)
        nc.scalar.dma_start(dmask[:], dmask_ext)
        nc.scalar.dma_start(w1sb[:], w1_ext)
        nc.scalar.dma_start(b1sb[:], b1_ext)
        nc.scalar.dma_start(w23sb[:], w23_ext)
        nc.scalar.dma_start(ecsb[:], ec_ext)
        nc.scalar.dma_start(xstg07[:, 4:6, :], x_src[:, 4:6, :])
        nc.scalar.dma_start(fc1sb[:], fc1_ext)
        nc.scalar.dma_start(fc2sb[:], fc2_ext)
        nc.scalar.dma_start(xstg07[:, 6:8, :], x_src[:, 6:8, :])

        # conv1 diag weights: d1[:, k, ch, :] = diag(w1[ch-block, k]);
        # high priority so the scheduler runs them on the idle early DVE
        # ahead of phase-A stats (they gate the whole PE stream)
        with tc.high_priority():
            for k in range(3):
                for ch in range(NCH):
                    nc.vector.tensor_scalar(d1sb[:, k, ch, :], dmask[:],
                                            w1sb[:, ch, k:k + 1], None,
                                            OP.mult)
        # keep the PE HAM clock gate warm before conv1 arrives; reads d1sb
        # (first DMA, lands ~3us), writes a throwaway psum tile
        wps = psum.tile([128, 128], F32, tag="warm", name="warm")
        if K_WARM and N_P >= 4:
            wrhs = d1sb[:, 0, :, :].rearrange("p a b -> p (a b)")[:, 0:QT]
            for i in range(K_WARM):
                nc.tensor.matmul(wps[:], d1sb[:, 0, 0, :], wrhs[:, 0:128],
                                 start=True, stop=True)

        def bridge_mm(t):
            # throwaway matmul gated on apply(t): keeps the PE HAM activity
            # window alive through the apply->transpose->scatter latency so
            # the clock gate doesn't drop back to half rate (~100ns each)
            nc.tensor.matmul(wps[:], d1sb[:, 0, 0, :], zX[:, t, 0:128],
                             start=True, stop=True)

        # ---- phase helpers ----
        def xf_src(t):
            return xstg07[:, t, :] if t < 8 else zX[:, t, :]

        def stats(t):
            xf = xf_src(t)
            if t in STATS_ACT:
                nc.scalar.activation(scr[:], xf, AF.Copy,
                                     accum_out=sums[:, t:t + 1])
                nc.scalar.activation(scr[:], xf, AF.Square,
                                     accum_out=sumsq[:, t:t + 1])
                nc.vector.tensor_scalar_mul(mu[:, t:t + 1],
                                            sums[:, t:t + 1], 1.0 / C)
                nc.vector.tensor_tensor(varv[:, t:t + 1], mu[:, t:t + 1],
                                        mu[:, t:t + 1], op=OP.mult)
                nc.vector.scalar_tensor_tensor(varv[:, t:t + 1],
                                               sumsq[:, t:t + 1],
                                               1.0 / C, varv[:, t:t + 1],
                                               OP.mult, OP.subtract)
            else:
                bs = pool.tile([128, 2, 6], F32, tag="bstats",
                               name=f"bs_{t}", bufs=4)
                nc.vector.bn_stats(bs[:, 0, :], xf[:, 0:512])
                nc.vector.bn_stats(bs[:, 1, :], xf[:, 512:1024])
                agg = pool.tile([128, 2], F32, tag="agg",
                                name=f"agg_{t}", bufs=4)
                nc.vector.bn_aggr(agg[:], bs[:])
                nc.vector.tensor_copy(mu[:, t:t + 1], agg[:, 0:1])
                nc.vector.tensor_copy(varv[:, t:t + 1], agg[:, 1:2])

        def finish_group_stats(g):
            gs = slice(4 * g, 4 * g + 4)
            nc.scalar.activation(varv[:, gs], varv[:, gs], AF.Sqrt,
                                 bias=epsb[:])
            nc.vector.reciprocal(rstd[:, gs], varv[:, gs])
            nc.vector.scalar_tensor_tensor(negmr[:, gs], mu[:, gs], -1.0,
                                           rstd[:, gs], OP.mult, OP.mult)

        def apply_ln(t):
            if t in APPLY_ACT:
                nc.scalar.activation(zX[:, t, :], xf_src(t), AF.Identity,
                                     scale=rstd[:, t:t + 1],
                                     bias=negmr[:, t:t + 1])
            else:
                nc.vector.tensor_scalar(zX[:, t, :], xf_src(t),
                                        mu[:, t:t + 1], rstd[:, t:t + 1],
                                        OP.subtract, OP.mult)

        xpose = [None] * 4

        def emit_scatter(g):
            for ch in range(NCH):
                dst = xB[:, ch, PAD + g * 512:PAD + (g + 1) * 512].rearrange(
                    "p (th t) -> p th t", th=4)
                src = sab(g)[:, :, ch, :]
                # on DVE (Pool's Q7 copy path measured ~6x slower); the ACT
                # queue must not sit between the transpose and the PE stream
                cp = nc.vector.tensor_copy(dst, src)
                add_dep_helper(cp.ins, xpose[g].ins, reason="xbar ordering")

        # ---- conv helpers ----
        def rsl(ch, a, b):
            return rall[:, ch, a:b]

        def conv1_pe(ch, q):
            lo, hi = W1[q], W1[q + 1]
            n = hi - lo
            ps1 = psum.tile([128, QT], F32, tag="c1ps", name=f"c1ps_{ch}_{q}")
            for k in range(3):
                off = PAD - 1 + k + lo
                nc.tensor.matmul(ps1[:, 0:n], d1sb[:, k, ch, :],
                                 xB[:, ch, off:off + n],
                                 start=(k == 0), stop=(k == 2))
            if ch % 2 == 0:
                nc.scalar.activation(rsl(ch, PAD + lo, PAD + hi), ps1[:, 0:n],
                                     AF.Relu, bias=b1sb[:, ch:ch + 1])
            else:
                nc.vector.tensor_scalar(rsl(ch, PAD + lo, PAD + hi),
                                        ps1[:, 0:n], b1sb[:, ch:ch + 1], 0.0,
                                        OP.add, OP.max)

        def conv1_lastcols(ch):
            # final 2 columns t=2046,2047 for PE channels (tiny DVE ops)
            sa = rsl(ch, PAD + 2046, PAD + 2048)
            xs = lambda d: xB[:, ch, PAD + 2046 + d:PAD + 2048 + d]
            nc.vector.tensor_scalar(sa, xs(-1), w1sb[:, ch, 0:1], None,
                                    OP.mult)
            nc.vector.scalar_tensor_tensor(sa, xs(0), w1sb[:, ch, 1:2], sa,
                                           OP.mult, OP.add)
            nc.vector.scalar_tensor_tensor(sa, xs(1), w1sb[:, ch, 2:3], sa,
                                           OP.mult, OP.add)
            nc.vector.tensor_scalar(sa, sa, b1sb[:, ch:ch + 1], 0.0,
                                    OP.add, OP.max)

        def conv1_chain(ch, q):
            lo = W1[q]
            hi = 2048 if q == 3 else W1[q + 1]
            a = rsl(ch, PAD + lo, PAD + hi)
            xs = lambda d: xB[:, ch, PAD + lo + d:PAD + hi + d]
            nc.vector.tensor_scalar(a, xs(-1), w1sb[:, ch, 0:1],
                                    b1sb[:, ch:ch + 1], OP.mult, OP.add)
            nc.vector.scalar_tensor_tensor(a, xs(0), w1sb[:, ch, 1:2], a,
                                           OP.mult, OP.add)
            nc.vector.scalar_tensor_tensor(a, xs(1), w1sb[:, ch, 2:3], a,
                                           OP.mult, OP.add)
            nc.vector.tensor_scalar_max(a, a, 0.0)

        def conv1_any(ch, q):
            if CH_ENG[ch] == 'P':
                conv1_pe(ch, q)
            else:
                conv1_chain(ch, q)

        d23_built = set()

        def build_d23(ch):
            if ch in d23_built:
                return
            d23_built.add(ch)
            # build this channel's 7 diag tap matrices on-device; ACT for
            # even channels, DVE for odd (both have slack mid-conv)
            for k in range(7):
                if ch % 2 == 0:
                    nc.scalar.activation(d23sb[:, k, ch, :], dmask[:],
                                         AF.Identity,
                                         scale=w23sb[:, ch, k:k + 1])
                else:
                    nc.vector.tensor_scalar(d23sb[:, k, ch, :], dmask[:],
                                            w23sb[:, ch, k:k + 1], None,
                                            OP.mult)

        def conv23_pair(ch, hh):
            # quarter pair (2hh, 2hh+1) in one 2-bank psum tile; k-outer so
            # each LDWEIGHTS feeds two 512-col matmuls
            build_d23(ch)
            qa, qb = 2 * hh, 2 * hh + 1
            pss = psum.tile([128, 1024], F32, tag="c23ps",
                            name=f"c23ps_{ch}_{hh}")
            for k in range(7):
                for j, q in enumerate((qa, qb)):
                    lo = W23[q]
                    off = PAD - 3 + k + lo
                    nc.tensor.matmul(pss[:, j * QT:(j + 1) * QT],
                                     d23sb[:, k, ch, :],
                                     rsl(ch, off, off + QT),
                                     start=(k == 0), stop=(k == 6))
            # edge-fix: the padded 7-tap composite differs from the true
            # conv3(conv2(.)) at the outermost columns; patch in PSUM
            if hh == 0:
                e = pss[:, 0:1]
                nc.vector.scalar_tensor_tensor(e, rsl(ch, PAD, PAD + 1),
                                               ecsb[:, ch, 0:1], e,
                                               OP.mult, OP.add)
                nc.vector.scalar_tensor_tensor(e, rsl(ch, PAD + 1, PAD + 2),
                                               ecsb[:, ch, 1:2], e,
                                               OP.mult, OP.add)
            else:
                e = pss[:, 1023:1024]
                nc.vector.scalar_tensor_tensor(e, rsl(ch, PAD + T - 2,
                                                      PAD + T - 1),
                                               ecsb[:, ch, 2:3], e,
                                               OP.mult, OP.add)
                nc.vector.scalar_tensor_tensor(e, rsl(ch, PAD + T - 1,
                                                      PAD + T),
                                               ecsb[:, ch, 3:4], e,
                                               OP.mult, OP.add)
            # c3 aliases xB (conv1 reads of the overlapped columns are
            # emitted earlier, so the WAR ordering holds)
            nc.scalar.activation(
                xB[:, ch, PAD + 1024 * hh:PAD + 1024 * (hh + 1)],
                pss[:], AF.Copy, accum_out=pools[:, ch, hh:hh + 1])
            nc.sync.dma_start(
                out=sba(hh)[:, ch, :, :],
                in_=xB[:, ch, PAD + 1024 * hh:PAD + 1024 * (hh + 1)],
                transpose=True)

        def edge_fix0():
            # batched t=0 edge patch for all channels (2 multiply + 2 add
            # ops instead of 16 per-pair PSUM patches); runs after all
            # pair-0 evacuations. The SE pool misses this ~1e-3 correction,
            # which shifts the sigmoid gate negligibly.
            dst = xB[:, :, PAD:PAD + 1]
            for j, c0 in enumerate((PAD, PAD + 1)):
                tmp = pool.tile([128, NCH, 1], F32, tag="efix",
                                name=f"efix0_{j}", bufs=2)
                nc.vector.tensor_tensor(tmp[:], rall[:, :, c0:c0 + 1],
                                        ecsb[:, :, j:j + 1], op=OP.mult)
                nc.vector.tensor_tensor(dst, dst, tmp[:], op=OP.add)

        def back_xpose0():
            for ch in range(NCH):
                nc.sync.dma_start(out=sba(0)[:, ch, :, :],
                                  in_=xB[:, ch, PAD:PAD + 1024],
                                  transpose=True)

        P_FIRST = sorted(range(NCH), key=lambda c: CH_ENG[c] != 'P')

        # ---- phase A + conv stream, readiness-ordered: scatter/conv for
        # group g-1 are emitted BEFORE group g's stats so the in-order ACT/
        # DVE queues run them as soon as their deps land, not after the next
        # group's x arrives ----
        for g in range(4):
            if g > 0:
                # high priority: when the scheduler has both these and the
                # next group's stats ready, feeding the PE wins
                with tc.high_priority(offset=4000):
                    emit_scatter(g - 1)
                    for ch in range(NCH):
                        conv1_any(ch, g - 1)
            if g == 3:
                # pair-0 conv23 overlaps the tail of phase A: channel ch only
                # needs its own conv1 q0-q2
                with tc.high_priority(offset=4000):
                    for ch in P_FIRST:
                        if CH_ENG[ch] == 'P':
                            conv23_pair(ch, 0)
            for t in range(4 * g, 4 * g + 4):
                # wait hint: the scheduler's DMA model thinks x lands much
                # earlier than it really does; without these it fills ACT/DVE
                # with later groups' stats ahead of the latency-critical
                # apply->transpose->scatter chain. Values ~= measured arrival.
                with tc.tile_wait_until(X_ARRIVE[t] / 1000):
                    stats(t)
            finish_group_stats(g)
            for t in range(4 * g, 4 * g + 4):
                apply_ln(t)
            if g > 0:
                with tc.high_priority(offset=4000):
                    for t in range(4 * g, 4 * g + min(K_BRIDGE, 4)):
                        bridge_mm(t)
            xpose[g] = nc.sync.dma_start(
                out=sab(g),
                in_=zX[:, 4 * g:4 * g + 4, :].rearrange("p a b -> p (a b)"),
                transpose=True)
        with tc.high_priority(offset=4000):
            emit_scatter(3)
            for ch in range(NCH):
                conv1_any(ch, 3)
                if CH_ENG[ch] == 'P':
                    conv1_lastcols(ch)

        # residual in (cast f32->bf16 SWDGE; overlaps the conv phase).
        # reuses zX; per-chunk WAR dep on that group's A->B transpose only.
        for q in range(8):
            nc.gpsimd.dma_start(zX[:, q * 2:(q + 1) * 2, :],
                                res_src[:, q * 2:(q + 1) * 2, :])

        for ch in range(NCH):
            if CH_ENG[ch] != 'P':
                conv23_pair(ch, 0)
        for ch in P_FIRST:
            conv23_pair(ch, 1)

        # ---- SE MLP ----
        h_ps = psum.tile([128, QT], F32, tag="c1ps", name="hps")[0:H, 0:2]
        for ch in range(NCH):
            nc.tensor.matmul(h_ps, fc1sb[:, ch, :], pools[:, ch, :],
                             start=(ch == 0), stop=(ch == NCH - 1))
        h_half = pool.tile([H, 2], F32, tag="h_half")
        nc.scalar.activation(h_half[:], h_ps, AF.Relu)
        h_sb = pool.tile([H, 1], F32, tag="hsb")
        nc.vector.tensor_reduce(h_sb[:], h_half[:], mybir.AxisListType.X,
                                OP.add)
        # replicate h across 128 cols, then one matmul pair computes
        # sigmoid(fc2^T h) broadcast over all partitions
        nc.vector.tensor_scalar_mul(h_rep[:], ones_h[:], h_sb[:])
        g_ps = psum.tile([128, 1024], F32, tag="c23ps", name="gps")
        nc.tensor.matmul(g_ps[:, 0:512], h_rep[:], fc2sb[:, 0:512],
                         start=True, stop=True)
        nc.tensor.matmul(g_ps[:, 512:1024], h_rep[:], fc2sb[:, 512:1024],
                         start=True, stop=True)
        nc.scalar.activation(gateA[:], g_ps[:], AF.Sigmoid)
        gateA3 = gateA[:].rearrange("p (ch c) -> p ch c", ch=NCH)

        # ---- output: gate mult + residual add + store, per 2-tile chunk ----
        for q in range(8):
            hf, to = q // 4, (2 * q) % 8
            eng = nc.gpsimd if TAIL_ENG[q] == 'G' else nc.vector
            for t in (2 * q, 2 * q + 1):
                st = sba(hf)[:, :, t % 8, :]
                eng.tensor_tensor(st, st, gateA3, op=OP.mult)
            zt = zX[:, 2 * q:2 * q + 2, :].rearrange(
                "p th (ch c) -> p ch th c", ch=NCH)
            eng.tensor_tensor(zt, zt, sba(hf)[:, :, to:to + 2, :],
                              op=OP.add)
            dq = nc.scalar if q % 2 == 0 else nc.sync
            dq.dma_start(out_dst[:, 2 * q:2 * q + 2, :],
                         zX[:, 2 * q:2 * q + 2, :])

    nc.compile()
    return nc


def _prep_weights(ln_w, ln_b, w1, w2, w3, fc1, fc2):
    import ml_dtypes
    w1 = w1[:, 0, :].astype(np.float64)   # [C, 3]
    w2 = w2[:, 0, :].astype(np.float64)   # [C, 5]
    w3 = w3[:, 0, :].astype(np.float64)   # [C, 3]
    ln_w = ln_w.astype(np.float64)
    ln_b = ln_b.astype(np.float64)
    w1f = w1 * ln_w[:, None]
    b1 = (ln_b * w1.sum(axis=1))

    def to_plh(a):  # [C, K] -> [128, NCH, K]
        return np.ascontiguousarray(
            a.reshape(NCH, 128, -1).transpose(1, 0, 2)).astype(np.float32)

    w1p = to_plh(w1f)
    b1p = np.ascontiguousarray(b1.reshape(NCH, 128).T).astype(np.float32)
    fc1p = to_plh((fc1.astype(np.float64) / T).T)
    fc2p = np.ascontiguousarray(
        fc2.astype(np.float64).T.reshape(H, NCH * 128)).astype(
            ml_dtypes.bfloat16)

    w23 = np.stack([np.convolve(w3[c], w2[c]) for c in range(C)])  # [C, 7]
    # edge-fix coefficients (negated: they accumulate into the psum)
    ec = np.stack([-w3[:, 0] * w2[:, 3], -w3[:, 0] * w2[:, 4],
                   -w3[:, 2] * w2[:, 0], -w3[:, 2] * w2[:, 1]], axis=1)  # [C,4]
    w23p = to_plh(w23)
    ecp = to_plh(ec)

    def diags(wk):  # [C, K] -> [128, K, NCH, 128] bf16 (partition-major)
        K = wk.shape[1]
        d = np.zeros((K, NCH, 128, 128), np.float32)
        for k in range(K):
            for chh in range(NCH):
                np.fill_diagonal(d[k, chh], wk[chh * 128:(chh + 1) * 128, k])
        return np.ascontiguousarray(
            d.transpose(2, 0, 1, 3)).astype(ml_dtypes.bfloat16)

    import ml_dtypes
    return {"w1p": w1p, "b1p": b1p, "w23p": w23p, "ecp": ecp,
            "fc1p": fc1p, "fc2p": fc2p,
            "dmask": np.eye(128, dtype=ml_dtypes.bfloat16)}


def kernel(x, residual_input, ln_w, ln_b, w1, w2, w3, fc1, fc2):
    x = np.asarray(x, dtype=np.float32)
    residual_input = np.asarray(residual_input, dtype=np.float32)
    wts = _prep_weights(np.asarray(ln_w), np.asarray(ln_b),
                        np.asarray(w1), np.asarray(w2), np.asarray(w3),
                        np.asarray(fc1), np.asarray(fc2))

    if "nc" not in _CACHE:
        _CACHE["nc"] = _build()
    nc = _CACHE["nc"]

    in_maps = []
    for b in range(B):
        m = {"x": np.ascontiguousarray(x[b]),
             "res": np.ascontiguousarray(residual_input[b])}
        m.update(wts)
        in_maps.append(m)
    res = run_bass_kernel_spmd(nc, in_maps, core_ids=list(range(N_CORES)))
    out = np.stack([res.results[i]["out"] for i in range(N_CORES)], axis=0)
    return out.astype(np.float32)


# revision 56
# speedup vs baseline: 1.1712x; 1.0018x over previous
"""Trainium2 Bass kernel for nn_AdapterBlock (LN -> dwconv x3 -> SE -> residual).

Data-parallel over batch: 8 samples -> 8 NeuronCores. v6: readiness-ordered
emission (all engine queues are in-order, so emission order IS the schedule).

Per core:
  - x loads f32 via the sync-engine HWDGE queue (~330 GB/s) into a 4-slot
    staging ring; the f32->bf16 cast is folded into the LN apply. The old
    casting SWDGE path ran at ~150 GB/s and gated the whole kernel.
  - LN stats per t-tile split ACT(accum)/DVE(bn_stats); apply split
    ACT(scale/bias form)/DVE(tensor_scalar) by tile parity, writing
    normalized bf16 into zX
  - xbar DMA-transpose to layout B per 4-tile group; scatter per (group,
    channel) so dep ranges stay inside one channel's row of xB
  - conv1 windows [0,510,1022,1534,2046): window q only needs groups <= q,
    so conv1 stage q is emitted right after scatter(q); final 2 columns
    patched by tiny DVE ops. Channel engines CH_ENG: 'P' diag-matmul on PE
    (evac split ACT/DVE by channel parity), 'H' chain on DVE
  - ~K_WARM dummy matmuls before conv1 warm the PE HAM clock gate
    (cold PE runs at half clock for 3.4us after any >3.4us idle)
  - conv23 on PE, k-outer over quarter-pairs in one 2-bank PSUM tile:
    7 LDWEIGHTS per channel-pair instead of 28, one paired evacuation
    (ACT, accum_out feeds SE pool) into contiguous c3h halves
  - one B->A transpose per half (DMA_TRANSPOSE has ~2us fixed cost)
  - gate broadcast: replicate h over 128 cols, single matmul pair against
    fc2 [H, C] bf16, sigmoid -> gateA
  - tail: out = c3*gateA + residual per 2-tile chunk on DVE; bf16 HWDGE
    store on scalar/sync queues, host upcasts to f32
"""

import os
import sys

sys.path.insert(0, "/opt/trn_rl_repo")

from contextlib import ExitStack

import numpy as np

import concourse.bass as bass  # noqa: F401
import concourse.bacc as bacc
import concourse.tile as tile
import concourse.mybir as mybir
from concourse.bass_utils import run_bass_kernel_spmd

B, T, C = 8, 2048, 1024
N_CORES = 8
NT = T // 128          # 16 t-tiles
NCH = C // 128         # 8 channel groups
H = C // 16            # SE hidden = 64
PAD = 4                # zero pad each side of the time axis (>= conv halo 3)
TF = T + 2 * PAD
QT = 512               # conv quarter
EPS = 1e-5

# conv1 windows: window q reads cols [W1[q]-1, W1[q+1]+1), chosen so it only
# needs scatter groups <= q; the last 2 columns are patched separately
W1 = [0, 510, 1022, 1534, 2046]
W23 = [0, 512, 1024, 1536, 2048]

F32 = mybir.dt.float32
BF16 = mybir.dt.bfloat16
AF = mybir.ActivationFunctionType
OP = mybir.AluOpType

# --- tunables ------------------------------------------------------------
# conv1 engine per channel-group: 'P' = TensorE diag-matmul, 'H' = DVE
# chain. conv23 is always PE. P channels must form a prefix.
CH_ENG = os.environ.get("K_CH_ENG", "PPPPPPHH")
STATS_ACT = set(int(x) for x in
                os.environ.get("K_STATS_ACT", "0,2,4,6,8,10,12,14").split(",")
                if x != "")
APPLY_ACT = set(int(x) for x in
                os.environ.get("K_APPLY_ACT", "0,2,4,6,8,10,12,14").split(",")
                if x != "")
K_WARM = int(os.environ.get("K_WARM", "12"))
K_BRIDGE = int(os.environ.get("K_BRIDGE", "4"))
TAIL_ENG = os.environ.get("K_TAIL_ENG", "VVVVVVVV")
N_P = len(CH_ENG) - len(CH_ENG.lstrip('P'))  # leading P channels

# measured x-tile arrival times (us): sync queue t0-3, scalar t4-7 (after
# small weights), gpsimd cast t8-15
X_ARRIVE = [13, 15, 19, 21, 22, 24, 28, 30,
            10, 12, 15, 17, 20, 23, 26, 29]

_CACHE = {}


def _build():
    nc = bacc.Bacc("TRN2", target_bir_lowering=False, debug=False,
                   num_devices=N_CORES)

    x_ext = nc.dram_tensor("x", [T, C], F32, kind="ExternalInput").ap()
    res_ext = nc.dram_tensor("res", [T, C], F32, kind="ExternalInput").ap()
    w1_ext = nc.dram_tensor("w1p", [128, NCH, 3], F32, kind="ExternalInput").ap()
    b1_ext = nc.dram_tensor("b1p", [128, NCH], F32, kind="ExternalInput").ap()
    ec_ext = nc.dram_tensor("ecp", [128, NCH, 4], F32, kind="ExternalInput").ap()
    w23_ext = nc.dram_tensor("w23p", [128, NCH, 7], F32, kind="ExternalInput").ap()
    dmask_ext = nc.dram_tensor("dmask", [128, 128], BF16, kind="ExternalInput").ap()
    fc1_ext = nc.dram_tensor("fc1p", [128, NCH, H], F32, kind="ExternalInput").ap()
    fc2_ext = nc.dram_tensor("fc2p", [H, NCH * 128], BF16, kind="ExternalInput").ap()
    out_ext = nc.dram_tensor("out", [T, C], BF16, kind="ExternalOutput").ap()

    x_src = x_ext.rearrange("(th p) c -> p th c", p=128)
    res_src = res_ext.rearrange("(th p) c -> p th c", p=128)
    out_dst = out_ext.rearrange("(th p) c -> p th c", p=128)

    with tile.TileContext(nc) as tc, ExitStack() as ctx:
        pool = ctx.enter_context(tc.tile_pool(name="main", bufs=1))
        from concourse.tile_rust import add_dep_helper

        # ---- weights (scalar HWDGE queue; d1 first, conv1 needs it ~13us)
        w1sb = pool.tile([128, NCH, 3], F32, tag="w1sb")
        b1sb = pool.tile([128, NCH], F32, tag="b1sb")
        ecsb = pool.tile([128, NCH, 4], F32, tag="ecsb")
        d1sb = pool.tile([128, 3, NCH, 128], BF16, tag="d1sb")
        fc1sb = pool.tile([128, NCH, H], F32, tag="fc1sb")
        fc2sb = pool.tile([H, NCH * 128], BF16, tag="fc2sb")
        d23sb = pool.tile([128, 7, NCH, 128], BF16, tag="d23sb")
        w23sb = pool.tile([128, NCH, 7], F32, tag="w23sb")
        dmask = pool.tile([128, 128], BF16, tag="dmask")


        # ---- buffers ----
        zX = pool.tile([128, NT, C], BF16, tag="zX")
        scr = pool.tile([128, C], BF16, tag="scr")
        sums = pool.tile([128, NT], F32, tag="sums")
        sumsq = pool.tile([128, NT], F32, tag="sumsq")
        mu = pool.tile([128, NT], F32, tag="mu")
        rstd = pool.tile([128, NT], F32, tag="rstd")
        negmr = pool.tile([128, NT], F32, tag="negmr")
        varv = pool.tile([128, NT], F32, tag="varv")
        epsb = pool.tile([128, 1], F32, tag="epsb")
        nc.vector.memset(epsb[:], EPS)
        # f32 x staging for tiles 0-7 (sync HWDGE loads in escalating chunk
        # sizes; no ring so the in-order sync queue never stalls on reuse).
        # Tiles 8-15 arrive via gpsimd cast-SWDGE straight into zX as bf16.
        xstg07 = pool.tile([128, 8, C], F32, tag="xstg07")
        # A->B stage ring; scatter trails one group so depth 2 suffices
        stgab = [pool.tile([128, 4 * C], BF16, tag="sab", name=f"sab{g}",
                           bufs=2)
                 for g in range(4)]
        stgba = [pool.tile([128, 8 * C], BF16, tag=f"sba{i}", name=f"sba{i}")
                 for i in range(2)]

        def sab(g):  # A->B view for 4-tile group g: [p, th(4), ch, t(128)]
            return stgab[g][:].rearrange("p (th ch t) -> p th ch t",
                                         th=4, ch=NCH)

        def sba(h):  # B->A view of half h: [p, ch, th(8), c(128)]
            return stgba[h][:].rearrange("p (ch th c) -> p ch th c",
                                         ch=NCH, th=8)
        xB = pool.tile([128, NCH, TF], BF16, tag="xB")
        nc.vector.memset(xB[:, :, 0:PAD], 0.0)
        nc.vector.memset(xB[:, :, PAD + T:TF], 0.0)
        rall = pool.tile([128, NCH, TF], BF16, tag="rall")
        nc.vector.memset(rall[:, :, 0:PAD], 0.0)
        nc.vector.memset(rall[:, :, PAD + T:TF], 0.0)
        pools = pool.tile([128, NCH, 2], F32, tag="pools")
        gateA = pool.tile([128, C], BF16, tag="gateA")
        h_rep = pool.tile([H, 128], BF16, tag="h_rep")
        ones_h = pool.tile([H, 128], BF16, tag="ones_h")
        nc.vector.memset(ones_h[:], 1.0)

        psum = ctx.enter_context(tc.tile_pool(name="ps", bufs=2, space="PSUM"))

        # ---- x loads + weights: each DMA queue tops out ~130-180 GB/s, so
        # x is spread over all three (sync f32 / scalar f32 / gpsimd cast)
        nc.sync.dma_start(xstg07[:, 0, :], x_src[:, 0, :])
        nc.sync.dma_start(xstg07[:, 1, :], x_src[:, 1, :])
        nc.sync.dma_start(xstg07[:, 2:4, :], x_src[:, 2:4, :])
        # tiles 8-15: cast f32->bf16 straight into zX on gpsimd SWDGE
        for c in range(4):
            t0 = 8 + 2 * c
            nc.gpsimd.dma_start(zX[:, t0:t0 + 2, :], x_src[:, t0:t0 + 2, :])
        # small weights first on scalar (the conv weights are built on-device
        # from these; the old 2.6MB diag-matrix loads ate the HBM ingest that
        # x needs in the first 30us)
        nc.scalar.dma_start(dmask[:], dmask_ext)
        nc.scalar.dma_start(w1sb[:], w1_ext)
        nc.scalar.dma_start(b1sb[:], b1_ext)
        nc.scalar.dma_start(w23sb[:], w23_ext)
        nc.scalar.dma_start(ecsb[:], ec_ext)
        nc.scalar.dma_start(xstg07[:, 4:6, :], x_src[:, 4:6, :])
        nc.scalar.dma_start(fc1sb[:], fc1_ext)
        nc.scalar.dma_start(fc2sb[:], fc2_ext)
        nc.scalar.dma_start(xstg07[:, 6:8, :], x_src[:, 6:8, :])

        # conv1 diag weights: d1[:, k, ch, :] = diag(w1[ch-block, k]);
        # high priority so the scheduler runs them on the idle early DVE
        # ahead of phase-A stats (they gate the whole PE stream)
        with tc.high_priority():
            for k in range(3):
                for ch in range(NCH):
                    nc.vector.tensor_scalar(d1sb[:, k, ch, :], dmask[:],
                                            w1sb[:, ch, k:k + 1], None,
                                            OP.mult)
        # keep the PE HAM clock gate warm before conv1 arrives; reads d1sb
        # (first DMA, lands ~3us), writes a throwaway psum tile
        wps = psum.tile([128, 128], F32, tag="warm", name="warm")
        if K_WARM and N_P >= 4:
            wrhs = d1sb[:, 0, :, :].rearrange("p a b -> p (a b)")[:, 0:QT]
            for i in range(K_WARM):
                nc.tensor.matmul(wps[:], d1sb[:, 0, 0, :], wrhs[:, 0:128],
                                 start=True, stop=True)

        def bridge_mm(t):
            # throwaway matmul gated on apply(t): keeps the PE HAM activity
            # window alive through the apply->transpose->scatter latency so
            # the clock gate doesn't drop back to half rate (~100ns each)
            nc.tensor.matmul(wps[:], d1sb[:, 0, 0, :], zX[:, t, 0:128],
                             start=True, stop=True)

        # ---- phase helpers ----
        def xf_src(t):
            return xstg07[:, t, :] if t < 8 else zX[:, t, :]

        def stats(t):
            xf = xf_src(t)
            if t in STATS_ACT:
                nc.scalar.activation(scr[:], xf, AF.Copy,
                                     accum_out=sums[:, t:t + 1])
                nc.scalar.activation(scr[:], xf, AF.Square,
                                     accum_out=sumsq[:, t:t + 1])
                nc.vector.tensor_scalar_mul(mu[:, t:t + 1],
                                            sums[:, t:t + 1], 1.0 / C)
                nc.vector.tensor_tensor(varv[:, t:t + 1], mu[:, t:t + 1],
                                        mu[:, t:t + 1], op=OP.mult)
                nc.vector.scalar_tensor_tensor(varv[:, t:t + 1],
                                               sumsq[:, t:t + 1],
                                               1.0 / C, varv[:, t:t + 1],
                                               OP.mult, OP.subtract)
            else:
                bs = pool.tile([128, 2, 6], F32, tag="bstats",
                               name=f"bs_{t}", bufs=4)
                nc.vector.bn_stats(bs[:, 0, :], xf[:, 0:512])
                nc.vector.bn_stats(bs[:, 1, :], xf[:, 512:1024])
                agg = pool.tile([128, 2], F32, tag="agg",
                                name=f"agg_{t}", bufs=4)
                nc.vector.bn_aggr(agg[:], bs[:])
                nc.vector.tensor_copy(mu[:, t:t + 1], agg[:, 0:1])
                nc.vector.tensor_copy(varv[:, t:t + 1], agg[:, 1:2])

        def finish_group_stats(g):
            gs = slice(4 * g, 4 * g + 4)
            nc.scalar.activation(varv[:, gs], varv[:, gs], AF.Sqrt,
                                 bias=epsb[:])
            nc.vector.reciprocal(rstd[:, gs], varv[:, gs])
            nc.vector.scalar_tensor_tensor(negmr[:, gs], mu[:, gs], -1.0,
                                           rstd[:, gs], OP.mult, OP.mult)

        def apply_ln(t):
            if t in APPLY_ACT:
                nc.scalar.activation(zX[:, t, :], xf_src(t), AF.Identity,
                                     scale=rstd[:, t:t + 1],
                                     bias=negmr[:, t:t + 1])
            else:
                nc.vector.tensor_scalar(zX[:, t, :], xf_src(t),
                                        mu[:, t:t + 1], rstd[:, t:t + 1],
                                        OP.subtract, OP.mult)

        xpose = [None] * 4

        def emit_scatter(g):
            for ch in range(NCH):
                dst = xB[:, ch, PAD + g * 512:PAD + (g + 1) * 512].rearrange(
                    "p (th t) -> p th t", th=4)
                src = sab(g)[:, :, ch, :]
                # on DVE (Pool's Q7 copy path measured ~6x slower); the ACT
                # queue must not sit between the transpose and the PE stream
                cp = nc.vector.tensor_copy(dst, src)
                add_dep_helper(cp.ins, xpose[g].ins, reason="xbar ordering")

        # ---- conv helpers ----
        def rsl(ch, a, b):
            return rall[:, ch, a:b]

        def conv1_pe(ch, q):
            lo, hi = W1[q], W1[q + 1]
            n = hi - lo
            ps1 = psum.tile([128, QT], F32, tag="c1ps", name=f"c1ps_{ch}_{q}")
            for k in range(3):
                off = PAD - 1 + k + lo
                nc.tensor.matmul(ps1[:, 0:n], d1sb[:, k, ch, :],
                                 xB[:, ch, off:off + n],
                                 start=(k == 0), stop=(k == 2))
            if ch % 2 == 0:
                nc.scalar.activation(rsl(ch, PAD + lo, PAD + hi), ps1[:, 0:n],
                                     AF.Relu, bias=b1sb[:, ch:ch + 1])
            else:
                nc.vector.tensor_scalar(rsl(ch, PAD + lo, PAD + hi),
                                        ps1[:, 0:n], b1sb[:, ch:ch + 1], 0.0,
                                        OP.add, OP.max)

        def conv1_lastcols(ch):
            # final 2 columns t=2046,2047 for PE channels (tiny DVE ops)
            sa = rsl(ch, PAD + 2046, PAD + 2048)
            xs = lambda d: xB[:, ch, PAD + 2046 + d:PAD + 2048 + d]
            nc.vector.tensor_scalar(sa, xs(-1), w1sb[:, ch, 0:1], None,
                                    OP.mult)
            nc.vector.scalar_tensor_tensor(sa, xs(0), w1sb[:, ch, 1:2], sa,
                                           OP.mult, OP.add)
            nc.vector.scalar_tensor_tensor(sa, xs(1), w1sb[:, ch, 2:3], sa,
                                           OP.mult, OP.add)
            nc.vector.tensor_scalar(sa, sa, b1sb[:, ch:ch + 1], 0.0,
                                    OP.add, OP.max)

        def conv1_chain(ch, q):
            lo = W1[q]
            hi = 2048 if q == 3 else W1[q + 1]
            a = rsl(ch, PAD + lo, PAD + hi)
            xs = lambda d: xB[:, ch, PAD + lo + d:PAD + hi + d]
            nc.vector.tensor_scalar(a, xs(-1), w1sb[:, ch, 0:1],
                                    b1sb[:, ch:ch + 1], OP.mult, OP.add)
            nc.vector.scalar_tensor_tensor(a, xs(0), w1sb[:, ch, 1:2], a,
                                           OP.mult, OP.add)
            nc.vector.scalar_tensor_tensor(a, xs(1), w1sb[:, ch, 2:3], a,
                                           OP.mult, OP.add)
            nc.vector.tensor_scalar_max(a, a, 0.0)

        def conv1_any(ch, q):
            if CH_ENG[ch] == 'P':
                conv1_pe(ch, q)
            else:
                conv1_chain(ch, q)

        d23_built = set()

        def build_d23(ch):
            if ch in d23_built:
                return
            d23_built.add(ch)
            # build this channel's 7 diag tap matrices on-device; ACT for
            # even channels, DVE for odd (both have slack mid-conv)
            for k in range(7):
                if ch % 2 == 0:
                    nc.scalar.activation(d23sb[:, k, ch, :], dmask[:],
                                         AF.Identity,
                                         scale=w23sb[:, ch, k:k + 1])
                else:
                    nc.vector.tensor_scalar(d23sb[:, k, ch, :], dmask[:],
                                            w23sb[:, ch, k:k + 1], None,
                                            OP.mult)

        def conv23_pair(ch, hh):
            # quarter pair (2hh, 2hh+1) in one 2-bank psum tile; k-outer so
            # each LDWEIGHTS feeds two 512-col matmuls
            build_d23(ch)
            qa, qb = 2 * hh, 2 * hh + 1
            pss = psum.tile([128, 1024], F32, tag="c23ps",
                            name=f"c23ps_{ch}_{hh}")
            for k in range(7):
                for j, q in enumerate((qa, qb)):
                    lo = W23[q]
                    off = PAD - 3 + k + lo
                    nc.tensor.matmul(pss[:, j * QT:(j + 1) * QT],
                                     d23sb[:, k, ch, :],
                                     rsl(ch, off, off + QT),
                                     start=(k == 0), stop=(k == 6))
            # edge-fix: the padded 7-tap composite differs from the true
            # conv3(conv2(.)) at the outermost columns; patch in PSUM
            if hh == 0:
                e = pss[:, 0:1]
                nc.vector.scalar_tensor_tensor(e, rsl(ch, PAD, PAD + 1),
                                               ecsb[:, ch, 0:1], e,
                                               OP.mult, OP.add)
                nc.vector.scalar_tensor_tensor(e, rsl(ch, PAD + 1, PAD + 2),
                                               ecsb[:, ch, 1:2], e,
                                               OP.mult, OP.add)
            else:
                e = pss[:, 1023:1024]
                nc.vector.scalar_tensor_tensor(e, rsl(ch, PAD + T - 2,
                                                      PAD + T - 1),
                                               ecsb[:, ch, 2:3], e,
                                               OP.mult, OP.add)
                nc.vector.scalar_tensor_tensor(e, rsl(ch, PAD + T - 1,
                                                      PAD + T),
                                               ecsb[:, ch, 3:4], e,
                                               OP.mult, OP.add)
            # c3 aliases xB (conv1 reads of the overlapped columns are
            # emitted earlier, so the WAR ordering holds)
            nc.scalar.activation(
                xB[:, ch, PAD + 1024 * hh:PAD + 1024 * (hh + 1)],
                pss[:], AF.Copy, accum_out=pools[:, ch, hh:hh + 1])
            nc.sync.dma_start(
                out=sba(hh)[:, ch, :, :],
                in_=xB[:, ch, PAD + 1024 * hh:PAD + 1024 * (hh + 1)],
                transpose=True)

        def edge_fix0():
            # batched t=0 edge patch for all channels (2 multiply + 2 add
            # ops instead of 16 per-pair PSUM patches); runs after all
            # pair-0 evacuations. The SE pool misses this ~1e-3 correction,
            # which shifts the sigmoid gate negligibly.
            dst = xB[:, :, PAD:PAD + 1]
            for j, c0 in enumerate((PAD, PAD + 1)):
                tmp = pool.tile([128, NCH, 1], F32, tag="efix",
                                name=f"efix0_{j}", bufs=2)
                nc.vector.tensor_tensor(tmp[:], rall[:, :, c0:c0 + 1],
                                        ecsb[:, :, j:j + 1], op=OP.mult)
                nc.vector.tensor_tensor(dst, dst, tmp[:], op=OP.add)

        def back_xpose0():
            for ch in range(NCH):
                nc.sync.dma_start(out=sba(0)[:, ch, :, :],
                                  in_=xB[:, ch, PAD:PAD + 1024],
                                  transpose=True)

        P_FIRST = sorted(range(NCH), key=lambda c: CH_ENG[c] != 'P')

        # ---- phase A + conv stream, readiness-ordered: scatter/conv for
        # group g-1 are emitted BEFORE group g's stats so the in-order ACT/
        # DVE queues run them as soon as their deps land, not after the next
        # group's x arrives ----
        for g in range(4):
            if g > 0:
                # high priority: when the scheduler has both these and the
                # next group's stats ready, feeding the PE wins
                with tc.high_priority(offset=4000):
                    emit_scatter(g - 1)
                    for ch in range(NCH):
                        conv1_any(ch, g - 1)
            if g == 3:
                # pair-0 conv23 overlaps the tail of phase A: channel ch only
                # needs its own conv1 q0-q2
                with tc.high_priority(offset=4000):
                    for ch in P_FIRST:
                        if CH_ENG[ch] == 'P':
                            conv23_pair(ch, 0)
            for t in range(4 * g, 4 * g + 4):
                # wait hint: the scheduler's DMA model thinks x lands much
                # earlier than it really does; without these it fills ACT/DVE
                # with later groups' stats ahead of the latency-critical
                # apply->transpose->scatter chain. Values ~= measured arrival.
                with tc.tile_wait_until(X_ARRIVE[t] / 1000):
                    stats(t)
            finish_group_stats(g)
            for t in range(4 * g, 4 * g + 4):
                apply_ln(t)
            if g > 0:
                with tc.high_priority(offset=4000):
                    for t in range(4 * g, 4 * g + min(K_BRIDGE, 4)):
                        bridge_mm(t)
            xpose[g] = nc.sync.dma_start(
                out=sab(g),
                in_=zX[:, 4 * g:4 * g + 4, :].rearrange("p a b -> p (a b)"),
                transpose=True)
        with tc.high_priority(offset=4000):
            emit_scatter(3)
            for ch in range(NCH):
                conv1_any(ch, 3)
                if CH_ENG[ch] == 'P':
                    conv1_lastcols(ch)

        # residual in (cast f32->bf16 SWDGE; overlaps the conv phase).
        # reuses zX; per-chunk WAR dep on that group's A->B transpose only.
        for q in range(8):
            nc.gpsimd.dma_start(zX[:, q * 2:(q + 1) * 2, :],
                                res_src[:, q * 2:(q + 1) * 2, :])

        for ch in range(NCH):
            if CH_ENG[ch] != 'P':
                conv23_pair(ch, 0)
        for ch in P_FIRST:
            conv23_pair(ch, 1)

        # ---- SE MLP ----
        h_ps = psum.tile([128, QT], F32, tag="c1ps", name="hps")[0:H, 0:2]
        for ch in range(NCH):
            nc.tensor.matmul(h_ps, fc1sb[:, ch, :], pools[:, ch, :],
                             start=(ch == 0), stop=(ch == NCH - 1))
        h_half = pool.tile([H, 2], F32, tag="h_half")
        nc.scalar.activation(h_half[:], h_ps, AF.Relu)
        h_sb = pool.tile([H, 1], F32, tag="hsb")
        nc.vector.tensor_reduce(h_sb[:], h_half[:], mybir.AxisListType.X,
                                OP.add)
        # replicate h across 128 cols, then one matmul pair computes
        # sigmoid(fc2^T h) broadcast over all partitions
        nc.vector.tensor_scalar_mul(h_rep[:], ones_h[:], h_sb[:])
        g_ps = psum.tile([128, 1024], F32, tag="c23ps", name="gps")
        nc.tensor.matmul(g_ps[:, 0:512], h_rep[:], fc2sb[:, 0:512],
                         start=True, stop=True)
        nc.tensor.matmul(g_ps[:, 512:1024], h_rep[:], fc2sb[:, 512:1024],
                         start=True, stop=True)
        nc.scalar.activation(gateA[:], g_ps[:], AF.Sigmoid)
        gateA3 = gateA[:].rearrange("p (ch c) -> p ch c", ch=NCH)

        # ---- output: gate mult + residual add + store, per 2-tile chunk ----
        for q in range(8):
            hf, to = q // 4, (2 * q) % 8
            eng = nc.gpsimd if TAIL_ENG[q] == 'G' else nc.vector
            for t in (2 * q, 2 * q + 1):
                st = sba(hf)[:, :, t % 8, :]
                eng.tensor_tensor(st, st, gateA3, op=OP.mult)
            zt = zX[:, 2 * q:2 * q + 2, :].rearrange(
                "p th (ch c) -> p ch th c", ch=NCH)
            eng.tensor_tensor(zt, zt, sba(hf)[:, :, to:to + 2, :],
                              op=OP.add)
            dq = nc.scalar if q % 2 == 0 else nc.sync
            dq.dma_start(out_dst[:, 2 * q:2 * q + 2, :],
                         zX[:, 2 * q:2 * q + 2, :])

    nc.compile()
    return nc


def _prep_weights(ln_w, ln_b, w1, w2, w3, fc1, fc2):
    import ml_dtypes
    w1 = w1[:, 0, :].astype(np.float64)   # [C, 3]
    w2 = w2[:, 0, :].astype(np.float64)   # [C, 5]
    w3 = w3[:, 0, :].astype(np.float64)   # [C, 3]
    ln_w = ln_w.astype(np.float64)
    ln_b = ln_b.astype(np.float64)
    w1f = w1 * ln_w[:, None]
    b1 = (ln_b * w1.sum(axis=1))

    def to_plh(a):  # [C, K] -> [128, NCH, K]
        return np.ascontiguousarray(
            a.reshape(NCH, 128, -1).transpose(1, 0, 2)).astype(np.float32)

    w1p = to_plh(w1f)
    b1p = np.ascontiguousarray(b1.reshape(NCH, 128).T).astype(np.float32)
    fc1p = to_plh((fc1.astype(np.float64) / T).T)
    fc2p = np.ascontiguousarray(
        fc2.astype(np.float64).T.reshape(H, NCH * 128)).astype(
            ml_dtypes.bfloat16)

    w23 = np.stack([np.convolve(w3[c], w2[c]) for c in range(C)])  # [C, 7]
    # edge-fix coefficients (negated: they accumulate into the psum)
    ec = np.stack([-w3[:, 0] * w2[:, 3], -w3[:, 0] * w2[:, 4],
                   -w3[:, 2] * w2[:, 0], -w3[:, 2] * w2[:, 1]], axis=1)  # [C,4]
    w23p = to_plh(w23)
    ecp = to_plh(ec)

    def diags(wk):  # [C, K] -> [128, K, NCH, 128] bf16 (partition-major)
        K = wk.shape[1]
        d = np.zeros((K, NCH, 128, 128), np.float32)
        for k in range(K):
            for chh in range(NCH):
                np.fill_diagonal(d[k, chh], wk[chh * 128:(chh + 1) * 128, k])
        return np.ascontiguousarray(
            d.transpose(2, 0, 1, 3)).astype(ml_dtypes.bfloat16)

    import ml_dtypes
    return {"w1p": w1p, "b1p": b1p, "w23p": w23p, "ecp": ecp,
            "fc1p": fc1p, "fc2p": fc2p,
            "dmask": np.eye(128, dtype=ml_dtypes.bfloat16)}


def kernel(x, residual_input, ln_w, ln_b, w1, w2, w3, fc1, fc2):
    x = np.asarray(x, dtype=np.float32)
    residual_input = np.asarray(residual_input, dtype=np.float32)
    wts = _prep_weights(np.asarray(ln_w), np.asarray(ln_b),
                        np.asarray(w1), np.asarray(w2), np.asarray(w3),
                        np.asarray(fc1), np.asarray(fc2))

    if "nc" not in _CACHE:
        _CACHE["nc"] = _build()
    nc = _CACHE["nc"]

    in_maps = []
    for b in range(B):
        m = {"x": np.ascontiguousarray(x[b]),
             "res": np.ascontiguousarray(residual_input[b])}
        m.update(wts)
        in_maps.append(m)
    res = run_bass_kernel_spmd(nc, in_maps, core_ids=list(range(N_CORES)))
    out = np.stack([res.results[i]["out"] for i in range(N_CORES)], axis=0)
    return out.astype(np.float32)
